# revision 5
# baseline (speedup 1.0000x reference)
"""Trainium2 Bass kernel for a 28-layer Qwen3-style decoder stack (single-token
decode with KV cache), tensor-parallel over 8 NeuronCores.

Sharding: per core c -> query heads {2c, 2c+1}, KV head c, MLP columns
[384c, 384(c+1)).  All matmuls run in float32r (12-bit-mantissa operands,
fp32 accumulate).  The residual stream h lives on-chip in fp32 as a
[128, 8] column tile; the two per-layer partial sums (o_proj, down_proj)
are combined with an 8-core AllReduce through HBM bounce buffers.

Per (core, layer) all weights + the KV cache slice are packed host-side
into one contiguous [128, BLOBW] fp32 block so each layer is a single
large DMA (~10 MB) that streams at near HBM roofline.
"""

import os
import sys

for _p in ("/opt/trn_rl_repo", "/root/.axon_site/_ro/trn_rl_repo"):
    if os.path.isdir(_p) and _p not in sys.path:
        sys.path.insert(0, _p)

import numpy as np

try:
    import jax

    jax.config.update("jax_compilation_cache_dir", "/tmp/jax_kernel_cache")
    jax.config.update("jax_persistent_cache_min_compile_time_secs", 2.0)
except Exception:
    pass

import concourse.bacc as bacc
import concourse.mybir as mybir
import concourse.tile as tile
from concourse import bass_utils

F32 = mybir.dt.float32
F32R = mybir.dt.float32r
AF = mybir.ActivationFunctionType

P = 8                      # cores
NL = int(os.environ.get("KNL_LAYERS", "28"))
REPEAT = int(os.environ.get("KNL_REPEAT", "1"))
NH, NKV, HD, D, FF = 16, 8, 128, 1024, 3072
NHL = NH // P              # 2 q heads per core
FFL = FF // P              # 384
CACHE = 2048
NPOS = CACHE + 1           # 2049
NCH = 17                   # position chunks of 128 (last = new token + pad)
EPS = 1e-6
SCALE = HD ** -0.5
LN_SCALE_BIAS = float(np.log(SCALE))

# blob column offsets (fp32 columns within the [128, BLOBW] per-layer block)
OFF_WQ = 0                  # 8 chunks x 256
OFF_WK = 2048               # 8 x 128
OFF_WV = 3072               # 8 x 128
OFF_WO = 4096               # 2 heads x 1024
OFF_WG = 6144               # 8 x 384
OFF_WU = 9216               # 8 x 384
OFF_WD = 12288              # 3 x 1024
OFF_V = 15360               # 16 chunks x 128  ([128 pos, 128 d] each)
OFF_KT = 17408              # K^T [128 d, 2048 pos]
OFF_KPAD = 19456            # [128, 128]: col 0 = k_new (device), rest zeros
OFF_LN = 19584              # [128, 16]: ln1 cols 0:8, ln2 cols 8:16
OFF_QKN = 19600             # row 0 only: qn (2x128) | kn (128)
BLOBW = 19984

_CACHED = {}


def _build_program():
    nc = bacc.Bacc("TRN2", target_bir_lowering=False, debug=False, num_devices=P)

    blob = nc.dram_tensor("blob", [NL, 128, BLOBW], F32R, kind="ExternalInput")
    h0 = nc.dram_tensor("h0", [128, 8], F32, kind="ExternalInput")
    ropeMT = nc.dram_tensor("ropeMT", [128, 128], F32R, kind="ExternalInput")
    ident = nc.dram_tensor("ident", [128, 128], F32, kind="ExternalInput")
    onesc2 = nc.dram_tensor("onesc2", [128, 2], F32R, kind="ExternalInput")
    onesr = nc.dram_tensor("onesr", [1, 128], F32R, kind="ExternalInput")
    sel34 = nc.dram_tensor("sel34", [34, 2], F32R, kind="ExternalInput")
    maskc = nc.dram_tensor("maskc", [128, 2 * NCH], F32, kind="ExternalInput")

    h_out = nc.dram_tensor("h_out", [128, 8], F32, kind="ExternalOutput")
    knew_out = nc.dram_tensor("knew_out", [NL, 1, 128], F32, kind="ExternalOutput")
    vnew_out = nc.dram_tensor("vnew_out", [NL, 1, 128], F32, kind="ExternalOutput")

    rg = [list(range(P))]

    with tile.TileContext(nc) as tc:
        with (
            tc.tile_pool(name="const", bufs=1) as cpool,
            tc.tile_pool(name="mega", bufs=2) as mpool,
            tc.tile_pool(name="work", bufs=2) as wpool,
            tc.tile_pool(name="ps", bufs=2, space="PSUM") as pp,
            tc.tile_pool(name="dram", bufs=4, space="DRAM") as dpool,
        ):
            # ---- persistent constants -------------------------------------
            mt = cpool.tile([128, 128], F32R, tag="mt")
            nc.sync.dma_start(mt[:], ropeMT[:])
            idt = cpool.tile([128, 128], F32, tag="idt")
            nc.sync.dma_start(idt[:], ident[:])
            oc2 = cpool.tile([128, 2], F32R, tag="oc2")
            nc.sync.dma_start(oc2[:], onesc2[:])
            orow = cpool.tile([1, 128], F32R, tag="orow")
            nc.sync.dma_start(orow[:], onesr[:])
            selt = cpool.tile([34, 2], F32R, tag="selt")
            nc.sync.dma_start(selt[:], sel34[:])
            mk = cpool.tile([128, 2 * NCH], F32, tag="mk")
            nc.sync.dma_start(mk[:], maskc[:])

            h = cpool.tile([128, 8], F32, tag="h")
            nc.sync.dma_start(h[:], h0[:])

            vpad = cpool.tile([128, 128], F32R, tag="vpad")
            nc.vector.memset(vpad[:].bitcast(F32), 0.0)
            sqk2 = cpool.tile([128, 2], F32R, tag="sqk2")
            nc.vector.memset(sqk2[:].bitcast(F32), 0.0)
            cb_eps = cpool.tile([1, 1], F32, tag="cb_eps")
            nc.vector.memset(cb_eps[:], EPS)
            cb_lns = cpool.tile([1, 1], F32, tag="cb_lns")
            nc.vector.memset(cb_lns[:], LN_SCALE_BIAS)

            def rms_cols(x_col, ln_slice, out, tagp):
                """out (fp32r [128,8]) = rms(x_col) * ln_slice."""
                junk = wpool.tile([128, 8], F32, tag=f"{tagp}_junk")
                rs = wpool.tile([128, 1], F32R, tag=f"{tagp}_rs")
                with nc.allow_low_precision(reason="fp32r"):
                    nc.scalar.activation(junk[:], x_col[:], AF.Square,
                                         accum_out=rs[:])
                tot = pp.tile([128, 2], F32, tag="pc2")
                nc.tensor.matmul(tot[0:1, 0:2], rs[:], oc2[:], start=True, stop=True)
                u = wpool.tile([1, 2], F32, tag=f"{tagp}_u")
                nc.scalar.activation(u[:], tot[0:1, 0:2], AF.Ln,
                                     bias=cb_eps[:], scale=1.0 / D)
                r = wpool.tile([1, 2], F32R, tag=f"{tagp}_r")
                nc.scalar.activation(r[:], u[:], AF.Exp, scale=-0.5)
                rb = pp.tile([128, 2], F32, tag="pc2")
                nc.tensor.matmul(rb[:], orow[:], r[:], start=True, stop=True)
                rb_sb = wpool.tile([128, 1], F32, tag=f"{tagp}_rb")
                nc.scalar.copy(rb_sb[:], rb[:, 0:1])
                tmp = wpool.tile([128, 8], F32, tag=f"{tagp}_tmp")
                nc.vector.tensor_scalar_mul(tmp[:], x_col[:], rb_sb[:])
                nc.vector.tensor_mul(out[:], tmp[:], ln_slice)

            # ---- layers (REPEAT>1 only for differential timing) -----------
            for rep in range(REPEAT):
              if rep > 0:
                nc.sync.dma_start(h[:], h0[:])
              for l in range(NL):
                  mega = mpool.tile([128, BLOBW], F32R, tag="mega")
                  nc.sync.dma_start(mega[:], blob[l])

                  LN1 = mega[:, OFF_LN:OFF_LN + 8].bitcast(F32)
                  LN2 = mega[:, OFF_LN + 8:OFF_LN + 16].bitcast(F32)
                  QN2 = mega[0:1, OFF_QKN:OFF_QKN + 256].bitcast(F32)
                  KN = mega[0:1, OFF_QKN + 256:OFF_QKN + 384].bitcast(F32)

                  # ---------- attention ----------
                  hs = wpool.tile([128, 8], F32R, tag="hs")
                  rms_cols(h, LN1, hs, "r1")

                  qkv = pp.tile([1, 512], F32, tag="row512")
                  for j in range(8):
                      st = j == 0
                      sp = j == 7
                      nc.tensor.matmul(
                          qkv[0:1, 0:256], hs[:, j:j + 1],
                          mega[:, OFF_WQ + 256 * j:OFF_WQ + 256 * (j + 1)],
                          start=st and True, stop=sp)
                      nc.tensor.matmul(
                          qkv[0:1, 256:384], hs[:, j:j + 1],
                          mega[:, OFF_WK + 128 * j:OFF_WK + 128 * (j + 1)],
                          start=False, stop=sp)
                      nc.tensor.matmul(
                          qkv[0:1, 384:512], hs[:, j:j + 1],
                          mega[:, OFF_WV + 128 * j:OFF_WV + 128 * (j + 1)],
                          start=False, stop=sp)

                  # v_new -> vpad row 0 (+ output)
                  nc.vector.tensor_copy(vpad[0:1, :], qkv[0:1, 384:512])
                  nc.gpsimd.dma_start(vnew_out[l], vpad[0:1, :].bitcast(F32))

                  # q/k * norm-weight (rows), transposes to columns
                  qnw = wpool.tile([1, 256], F32, tag="qnw")
                  nc.vector.tensor_mul(qnw[:], qkv[0:1, 0:256], QN2)
                  knw = wpool.tile([1, 128], F32, tag="knw")
                  nc.vector.tensor_mul(knw[:], qkv[0:1, 256:384], KN)

                  qT = pp.tile([128, 2], F32, tag="pc2")
                  nc.tensor.transpose(qT[:, 0:1], qnw[0:1, 0:128], idt[0:1, 0:1])
                  nc.tensor.transpose(qT[:, 1:2], qnw[0:1, 128:256], idt[0:1, 0:1])
                  q_colr = wpool.tile([128, 2], F32R, tag="q_colr")
                  nc.vector.tensor_copy(q_colr[:], qT[:])
                  kT = pp.tile([128, 2], F32, tag="pc2")
                  nc.tensor.transpose(kT[:, 0:1], knw[:], idt[0:1, 0:1])
                  k_colr = wpool.tile([128, 2], F32R, tag="k_colr")
                  nc.vector.tensor_copy(k_colr[:, 0:1], kT[:, 0:1])
                  nc.vector.tensor_copy(k_colr[:, 1:2], sqk2[:, 1:2])

                  # per-head inverse-rms scales (q: with SCALE folded in)
                  sq = wpool.tile([128, 2], F32R, tag="sq")
                  nc.vector.tensor_mul(sq[:], q_colr[:], q_colr[:])
                  qsum = pp.tile([128, 2], F32, tag="pc2")
                  nc.tensor.matmul(qsum[0:1, 0:2], oc2[:, 0:1], sq[:],
                                   start=True, stop=True)
                  uq = wpool.tile([1, 2], F32, tag="uq")
                  nc.scalar.activation(uq[:], qsum[0:1, 0:2], AF.Ln,
                                       bias=cb_eps[:], scale=1.0 / HD)
                  sq_r = wpool.tile([1, 2], F32R, tag="sq_r")
                  nc.scalar.activation(sq_r[:], uq[:], AF.Exp, scale=-0.5,
                                       bias=cb_lns[:])
                  sk = wpool.tile([128, 2], F32R, tag="sk")
                  nc.vector.tensor_mul(sk[:], k_colr[:], k_colr[:])
                  ksum = pp.tile([128, 2], F32, tag="pc2")
                  nc.tensor.matmul(ksum[0:1, 0:2], oc2[:, 0:1], sk[:],
                                   start=True, stop=True)
                  uk = wpool.tile([1, 2], F32, tag="uk")
                  nc.scalar.activation(uk[:], ksum[0:1, 0:2], AF.Ln,
                                       bias=cb_eps[:], scale=1.0 / HD)
                  sk_r = wpool.tile([1, 2], F32R, tag="sk_r")
                  nc.scalar.activation(sk_r[:], uk[:], AF.Exp, scale=-0.5)

                  # rope + per-head scale broadcast (q chain, then k chain)
                  qrope = pp.tile([128, 2], F32, tag="pc2")
                  nc.tensor.matmul(qrope[:], mt[:], q_colr[:], start=True, stop=True)
                  bq = pp.tile([128, 2], F32, tag="pc2")
                  nc.tensor.matmul(bq[:], orow[:], sq_r[:], start=True, stop=True)
                  bq_sb = wpool.tile([128, 2], F32, tag="bq_sb")
                  nc.scalar.copy(bq_sb[:], bq[:])
                  q_fin = wpool.tile([128, 2], F32R, tag="q_fin")
                  nc.vector.tensor_mul(q_fin[:], qrope[:], bq_sb[:])

                  krope = pp.tile([128, 2], F32, tag="pc2")
                  nc.tensor.matmul(krope[:], mt[:], k_colr[:], start=True, stop=True)
                  bk = pp.tile([128, 2], F32, tag="pc2")
                  nc.tensor.matmul(bk[:], orow[:], sk_r[:], start=True, stop=True)
                  bk_sb = wpool.tile([128, 2], F32, tag="bk_sb")
                  nc.scalar.copy(bk_sb[:], bk[:])
                  # k_new: into the kpad region of the mega tile (col 0) + fp32 copy
                  nc.vector.tensor_mul(mega[:, OFF_KPAD:OFF_KPAD + 1],
                                       krope[:, 0:1], bk_sb[:, 0:1])
                  k_keep = wpool.tile([128, 1], F32, tag="k_keep")
                  nc.vector.tensor_mul(k_keep[:], krope[:, 0:1], bk_sb[:, 0:1])

                  # k_new output (transpose to a row)
                  knT = pp.tile([1, 128], F32, tag="pc2")
                  nc.tensor.transpose(knT[:], k_keep[:], idt[:])
                  kn_row = wpool.tile([1, 128], F32, tag="kn_row")
                  nc.scalar.copy(kn_row[:], knT[:])
                  nc.gpsimd.dma_start(knew_out[l], kn_row[:])

                  # scores: 17 chunks -> psum [128, 34]
                  sc = pp.tile([128, 2 * NCH], F32, tag="pscore")
                  for c in range(NCH):
                      base = OFF_KT + 128 * c if c < 16 else OFF_KPAD
                      nc.tensor.matmul(sc[:, 2 * c:2 * c + 2],
                                       mega[:, base:base + 128], q_fin[:],
                                       start=True, stop=True)
                  scm = wpool.tile([128, 2 * NCH], F32, tag="scm")
                  nc.vector.tensor_add(scm[:], sc[:], mk[:])
                  ex = wpool.tile([128, 2 * NCH], F32R, tag="ex")
                  nc.scalar.activation(ex[:], scm[:], AF.Exp)

                  # softmax denominators -> reciprocal row [1, 2]
                  cs = pp.tile([34, 2], F32, tag="pc2")
                  nc.tensor.matmul(cs[:], ex[:], oc2[:], start=True, stop=True)
                  cs_sb = wpool.tile([34, 1], F32R, tag="cs_sb")
                  nc.scalar.copy(cs_sb[:], cs[:, 0:1])
                  s2 = pp.tile([128, 2], F32, tag="pc2")
                  nc.tensor.matmul(s2[0:1, 0:2], cs_sb[:], selt[:],
                                   start=True, stop=True)
                  rec = wpool.tile([1, 2], F32R, tag="rec")
                  with nc.allow_low_precision(reason="fp32r"):
                      nc.vector.reciprocal(rec[:], s2[0:1, 0:2])

                  # attention-weighted V: accumulate [2, 128]
                  attn = pp.tile([2, 128], F32, tag="big")
                  for c in range(NCH):
                      rhs = (mega[:, OFF_V + 128 * c:OFF_V + 128 * (c + 1)]
                             if c < 16 else vpad[:])
                      nc.tensor.matmul(attn[:], ex[:, 2 * c:2 * c + 2], rhs,
                                       start=(c == 0), stop=(c == NCH - 1))
                  attn_sb = wpool.tile([2, 128], F32, tag="attn_sb")
                  nc.scalar.copy(attn_sb[:], attn[:])
                  atT = pp.tile([128, 2], F32, tag="pc2")
                  nc.tensor.transpose(atT[:], attn_sb[:], idt[0:2, 0:2])
                  b2 = pp.tile([128, 2], F32, tag="pc2")
                  nc.tensor.matmul(b2[:], orow[:], rec[:], start=True, stop=True)
                  b2_sb = wpool.tile([128, 2], F32, tag="b2_sb")
                  nc.scalar.copy(b2_sb[:], b2[:])
                  at3 = wpool.tile([128, 3], F32R, tag="at3")
                  nc.vector.tensor_mul(at3[:, 0:2], atT[:], b2_sb[:])
                  nc.vector.tensor_copy(at3[:, 2:3], sqk2[:, 0:1])

                  # o_proj (column orientation, garbage-column trick)
                  op = pp.tile([128, 16], F32, tag="big")
                  for oc in range(8):
                      for hh in range(2):
                          base = OFF_WO + hh * 1024 + oc * 128
                          nc.tensor.matmul(op[:, 2 * oc:2 * oc + 2],
                                           mega[:, base:base + 128],
                                           at3[:, hh:hh + 2],
                                           start=(hh == 0), stop=(hh == 1))
                  osum = wpool.tile([128, 8], F32, tag="osum")
                  opv = op[:].rearrange("p (a b) -> p a b", b=2)
                  nc.scalar.copy(osum[:].rearrange("p (a b) -> p a b", b=1),
                                 opv[:, :, 0:1])

                  # all-reduce attention delta; h += delta
                  cc_in = dpool.tile([128, 8], F32, tag="cc_in")
                  cc_out = dpool.tile([128, 8], F32, tag="cc_out")
                  nc.scalar.dma_start(cc_in[:], osum[:])
                  nc.gpsimd.collective_compute(
                      "AllReduce", mybir.AluOpType.add, ins=[cc_in.opt()],
                      outs=[cc_out.opt()], replica_groups=rg)
                  odelta = wpool.tile([128, 8], F32, tag="odelta")
                  nc.scalar.dma_start(odelta[:], cc_out[:])
                  nc.vector.tensor_add(h[:], h[:], odelta[:])

                  # ---------- MLP ----------
                  hs2 = wpool.tile([128, 8], F32R, tag="hs2")
                  rms_cols(h, LN2, hs2, "r2")

                  g_ps = pp.tile([1, 384], F32, tag="row512")
                  u_ps = pp.tile([1, 384], F32, tag="row512")
                  for j in range(8):
                      nc.tensor.matmul(
                          g_ps[:], hs2[:, j:j + 1],
                          mega[:, OFF_WG + 384 * j:OFF_WG + 384 * (j + 1)],
                          start=(j == 0), stop=(j == 7))
                      nc.tensor.matmul(
                          u_ps[:], hs2[:, j:j + 1],
                          mega[:, OFF_WU + 384 * j:OFF_WU + 384 * (j + 1)],
                          start=(j == 0), stop=(j == 7))
                  g_sb = wpool.tile([1, 384], F32, tag="g_sb")
                  nc.scalar.copy(g_sb[:], g_ps[:])
                  u_sb = wpool.tile([1, 384], F32, tag="u_sb")
                  nc.scalar.copy(u_sb[:], u_ps[:])

                  gcol = pp.tile([128, 3], F32, tag="pc2")
                  ucol = pp.tile([128, 3], F32, tag="pc2")
                  for j in range(3):
                      nc.tensor.transpose(gcol[:, j:j + 1],
                                          g_sb[0:1, 128 * j:128 * (j + 1)],
                                          idt[0:1, 0:1])
                      nc.tensor.transpose(ucol[:, j:j + 1],
                                          u_sb[0:1, 128 * j:128 * (j + 1)],
                                          idt[0:1, 0:1])
                  # silu(g) * u, in columns
                  eneg = wpool.tile([128, 3], F32, tag="eneg")
                  nc.scalar.activation(eneg[:], gcol[:], AF.Exp, scale=-1.0)
                  ep1 = wpool.tile([128, 3], F32, tag="ep1")
                  nc.vector.tensor_scalar_add(ep1[:], eneg[:], 1.0)
                  sig = wpool.tile([128, 3], F32, tag="sig")
                  nc.vector.reciprocal(sig[:], ep1[:])
                  gs = wpool.tile([128, 3], F32, tag="gs")
                  nc.vector.tensor_mul(gs[:], gcol[:], sig[:])
                  act4 = wpool.tile([128, 4], F32R, tag="act4")
                  nc.vector.tensor_mul(act4[:, 0:3], ucol[:], gs[:])
                  nc.vector.tensor_copy(act4[:, 3:4], sqk2[:, 0:1])

                  # down_proj (column orientation)
                  dp = pp.tile([128, 16], F32, tag="big")
                  for oc in range(8):
                      for j in range(3):
                          base = OFF_WD + j * 1024 + oc * 128
                          nc.tensor.matmul(dp[:, 2 * oc:2 * oc + 2],
                                           mega[:, base:base + 128],
                                           act4[:, j:j + 2],
                                           start=(j == 0), stop=(j == 2))
                  dsum = wpool.tile([128, 8], F32, tag="dsum")
                  dpv = dp[:].rearrange("p (a b) -> p a b", b=2)
                  nc.scalar.copy(dsum[:].rearrange("p (a b) -> p a b", b=1),
                                 dpv[:, :, 0:1])

                  cc_in2 = dpool.tile([128, 8], F32, tag="cc_in")
                  cc_out2 = dpool.tile([128, 8], F32, tag="cc_out")
                  nc.scalar.dma_start(cc_in2[:], dsum[:])
                  nc.gpsimd.collective_compute(
                      "AllReduce", mybir.AluOpType.add, ins=[cc_in2.opt()],
                      outs=[cc_out2.opt()], replica_groups=rg)
                  ddelta = wpool.tile([128, 8], F32, tag="ddelta")
                  nc.scalar.dma_start(ddelta[:], cc_out2[:])
                  nc.vector.tensor_add(h[:], h[:], ddelta[:])

            nc.sync.dma_start(h_out[:], h[:])
    nc.compile()
    return nc


def _pack_inputs(hidden_states, k_caches, v_caches, position_cos, position_sin,
                 attention_mask, ln1_w, ln2_w, qn_w, kn_w, w_q, w_k, w_v, w_o,
                 w_gate, w_up, w_down):
    f = np.float32
    cos = np.asarray(position_cos, f).reshape(HD)
    sin = np.asarray(position_sin, f).reshape(HD)
    mask = np.asarray(attention_mask, f).reshape(NPOS)

    M = np.zeros((HD, HD), f)
    M[np.arange(HD), np.arange(HD)] = cos
    M[np.arange(64), np.arange(64) + 64] = -sin[:64]
    M[np.arange(64) + 64, np.arange(64)] = sin[64:]
    ropeMT = np.ascontiguousarray(M.T)

    maskc = np.full((128, 2 * NCH), -1e30, f)
    for c in range(NCH):
        n = 128 if c < 16 else 1
        maskc[:n, 2 * c] = mask[128 * c:128 * c + n]
        maskc[:n, 2 * c + 1] = mask[128 * c:128 * c + n]

    sel = np.zeros((34, 2), f)
    sel[0::2, 0] = 1.0
    sel[1::2, 1] = 1.0

    h0 = np.ascontiguousarray(np.asarray(hidden_states, f).reshape(8, 128).T)

    wq = np.asarray(w_q, f)
    wk = np.asarray(w_k, f)
    wv = np.asarray(w_v, f)
    wo = np.asarray(w_o, f)
    wg = np.asarray(w_gate, f)
    wu = np.asarray(w_up, f)
    wd = np.asarray(w_down, f)
    kc = np.asarray(k_caches, f)
    vc = np.asarray(v_caches, f)
    ln1 = np.asarray(ln1_w, f)
    ln2 = np.asarray(ln2_w, f)
    qn = np.asarray(qn_w, f)
    kn = np.asarray(kn_w, f)

    blobs = []
    for c in range(P):
        b = np.zeros((NL, 128, BLOBW), f)
        for l in range(NL):
            b[l, :, OFF_WQ:OFF_WQ + 2048] = (
                wq[l, :, 256 * c:256 * (c + 1)].reshape(8, 128, 256)
                .transpose(1, 0, 2).reshape(128, 2048))
            b[l, :, OFF_WK:OFF_WK + 1024] = (
                wk[l, :, 128 * c:128 * (c + 1)].reshape(8, 128, 128)
                .transpose(1, 0, 2).reshape(128, 1024))
            b[l, :, OFF_WV:OFF_WV + 1024] = (
                wv[l, :, 128 * c:128 * (c + 1)].reshape(8, 128, 128)
                .transpose(1, 0, 2).reshape(128, 1024))
            b[l, :, OFF_WO:OFF_WO + 2048] = (
                wo[l, 256 * c:256 * (c + 1), :].reshape(2, 128, 1024)
                .transpose(1, 0, 2).reshape(128, 2048))
            b[l, :, OFF_WG:OFF_WG + 3072] = (
                wg[l, :, FFL * c:FFL * (c + 1)].reshape(8, 128, FFL)
                .transpose(1, 0, 2).reshape(128, 3072))
            b[l, :, OFF_WU:OFF_WU + 3072] = (
                wu[l, :, FFL * c:FFL * (c + 1)].reshape(8, 128, FFL)
                .transpose(1, 0, 2).reshape(128, 3072))
            b[l, :, OFF_WD:OFF_WD + 3072] = (
                wd[l, FFL * c:FFL * (c + 1), :].reshape(3, 128, 1024)
                .transpose(1, 0, 2).reshape(128, 3072))
            b[l, :, OFF_V:OFF_V + 2048] = (
                vc[l, c].reshape(16, 128, 128)
                .transpose(1, 0, 2).reshape(128, 2048))
            b[l, :, OFF_KT:OFF_KT + 2048] = kc[l, c].T
            b[l, :, OFF_LN:OFF_LN + 8] = ln1[l].reshape(8, 128).T
            b[l, :, OFF_LN + 8:OFF_LN + 16] = ln2[l].reshape(8, 128).T
            b[l, 0, OFF_QKN:OFF_QKN + 128] = qn[l]
            b[l, 0, OFF_QKN + 128:OFF_QKN + 256] = qn[l]
            b[l, 0, OFF_QKN + 256:OFF_QKN + 384] = kn[l]
        blobs.append(b)

    common = {
        "h0": h0,
        "ropeMT": ropeMT,
        "ident": np.eye(128, dtype=f),
        "onesc2": np.ones((128, 2), f),
        "onesr": np.ones((1, 128), f),
        "sel34": sel,
        "maskc": maskc,
    }
    return [{**common, "blob": blobs[c]} for c in range(P)]


def kernel(**inputs):
    if "nc" not in _CACHED:
        _CACHED["nc"] = _build_program()
    nc = _CACHED["nc"]

    in_maps = _pack_inputs(**inputs)
    trace = bool(int(os.environ.get("KNL_TRACE", "0")))
    res = bass_utils.run_bass_kernel_spmd(
        nc, in_maps, core_ids=list(range(P)), trace=trace)
    if trace:
        _CACHED["exec_time_ns"] = res.exec_time_ns
        _CACHED["results"] = res

    h = res.results[0]["h_out"].T.reshape(1, 1, D).astype(np.float32)

    kc = np.asarray(inputs["k_caches"], np.float32)
    vc = np.asarray(inputs["v_caches"], np.float32)
    k_new = np.stack([res.results[c]["knew_out"][:, 0, :] for c in range(P)],
                     axis=1)  # [NL, 8, 128]
    v_new = np.stack([res.results[c]["vnew_out"][:, 0, :] for c in range(P)],
                     axis=1)
    k_out = np.concatenate([kc[:NL], k_new[:, :, None, :]], axis=2)
    v_out = np.concatenate([vc[:NL], v_new[:, :, None, :]], axis=2)
    if NL < 28:
        return h, k_out, v_out
    return h, k_out, v_out
